# revision 69
# baseline (speedup 1.0000x reference)
"""BiMamba Trainium2 kernel.

Sharding: 8 cores = (batch 2) x (direction 2) x (head-half 2). Each core runs an
identical SPMD Bass program on its slice: x[b]^T (time-flipped for bwd), in_proj
rows for its 12 heads (+ shared B/C rows). Per-core output: unnormalized
projected partial (2048, 768) bf16 + partial sum-of-squares. The RMSNorm rsqrt
commutes with the linear projection, so the host applies it to the summed
partials, then adds proj bias.

Scan: chunked SSD, chunk=128:
  y_t = sum_{s<=t} (B_s.C_t) exp(Acum_t - Acum_s) dt_s x_s + exp(Acum_t) (C_t.h_prev)
Decay matrix: exact-fp32 cumsum via triangular matmul; the (s,t) plane
D[s,t] = (logdt_s - Acum_s) + Acum_t is built by one K=39 bf16 matmul per chunk
(3 ones rows x 3-way-split Acum_t + 36 blocked-ones rows x 3-way-split per-head
bias, rows ordered h-major for single-DMA staging); Pool min-clamp; ACT exp.

vs. baseline rewrite: whole-TB decay staging (4 HWDGE DMAs/TB), silu/exp ACT
ops grouped to 2 act-table loads per TB, ws/aend broadcasts read from lall col
127 via AP, state decay folded into the pst matmul accumulation as scaled-
identity matmuls, Dp residual folded into the py accumulation as Dp*I matmuls,
bf16 scan operands (sub-256-col matmuls run 4x faster in bf16).
"""
import os
import numpy as np
from contextlib import ExitStack

import bass_rust
import concourse.bass as bass
import concourse.tile as tile
from concourse import bacc, mybir
from concourse.bass_utils import run_bass_kernel_spmd
from concourse.masks import make_identity


def _after(insts, *deps):
    """nosync (scheduler-only) ordering: each of `insts` runs after all `deps`.
    Used to keep silu-table and exp-table ACT ops grouped so the post-schedule
    act-table-load pass inserts ~2 loads per time block instead of ~11."""
    ds = bass_rust.InstructionNameOrderedSet()
    for d in deps:
        if d is not None:
            ds.add(d.ins.name)
    if len(ds):
        for i in insts:
            i.ins.add_nosync_dependencies_from(ds)

FP32 = mybir.dt.float32
FP32R = mybir.dt.float32r
BF16 = mybir.dt.bfloat16
AF = mybir.ActivationFunctionType
ALU = mybir.AluOpType

D_MODEL = 768
D_STATE = 16
HEADDIM = 64
D_CONV = 4
SEQ = 2048
NH = 12                  # heads per core
HH = NH * HEADDIM        # 768 x-channels per core
CMJ = HH + NH + 2 * D_STATE   # 812 c-major feats: [x 768 | dt 12 | B 16 | C 16]
TMJ = HH + NH            # 780 t-major feats: [z 768 | dt 12]
CH = 128
NCHUNK = SEQ // CH       # 16
TB = 256                 # time block
NTB = SEQ // TB
CPB = TB // CH           # 2
NKT = 6                  # d_model k-tiles
EPS = 1e-5
P = 128
KD = 3 + 3 * NH          # 39 decay-matmul contraction rows


def _rep(ap_tile, inner, outer_count, inner_count, outer_step, inner_step):
    """free-pattern AP helper on a 2D tile: [[pstep,P],[outer],[inner]]"""
    return bass.AP(tensor=ap_tile.tensor, offset=ap_tile.offset,
                   ap=[[ap_tile.ap[0][0], ap_tile.ap[0][1]],
                       [outer_step, outer_count], [inner_step, inner_count]])


def _pbcast(src, parts):
    """broadcast a (1, N) AP across `parts` partitions (DMA source only)."""
    assert src.ap[0][1] == 1
    return bass.AP(tensor=src.tensor, offset=src.offset,
                   ap=[src.ap[0], [0, parts]] + [list(d) for d in src.ap[1:]])


def build_program():
    nc = bacc.Bacc("TRN2", target_bir_lowering=False, debug=False, num_devices=8)

    def din(name, shape, dt=FP32):
        return nc.dram_tensor(name, shape, dt, kind="ExternalInput").ap()

    # packed weights, p-major so each loads with ONE contiguous DMA
    NWX = NKT * HH                  # 4608 bf16 x-rows of Wc
    NCT = NKT * 44                  # 264 fp32r dt|B|C rows of Wc
    NWT = NKT * HH                  # 4608 bf16 (z rows only)
    NDG = D_CONV * NKT * P + 2 * D_CONV * D_STATE   # 3200 bf16
    NWB = NKT * D_MODEL + NH * P    # 6144 bf16: wcomb | dpid
    d_xT = din("xT", (D_MODEL, SEQ), FP32R)
    d_xTb = din("xTb", (D_MODEL, SEQ), BF16)
    d_WCX = din("WCX", (P, NWX), BF16)
    d_WCT = din("WCT", (P, NCT), FP32R)
    d_WTP = din("WTP", (P, NWT), BF16)
    d_DGP = din("DGP", (P, NDG), BF16)
    d_WBP = din("WBP", (P, NWB), BF16)
    d_CONVBX = din("CONVBX", (P, NKT))                      # x-part conv bias per c-tile
    d_CONVBB = din("CONVBB", (D_STATE, 1))
    d_CONVBC = din("CONVBC", (D_STATE, 1))
    d_DTBIAS = din("DTBIAS", (NH, 1))
    d_ANEG = din("ANEG", (NH, 1))
    d_ANEG_BC = din("ANEG_BC", (P, NH))
    d_TRI = din("TRI", (P, P))                              # tri[s,t]=1 if s<=t
    d_ONES3 = din("ONES3", (3, TB), BF16)
    d_RHSC = din("RHSC", (3 * NH, CPB * NH * CH), BF16)     # blocked ones, rows h*3+j
    d_EYEREP = din("EYEREP", (D_STATE, NH * D_STATE), BF16)  # delta(n,m) per head
    d_OUT1 = nc.dram_tensor("OUT1", (SEQ, D_MODEL), BF16, kind="ExternalOutput").ap()
    d_OUT2 = nc.dram_tensor("OUT2", (P, NCHUNK), FP32, kind="ExternalOutput").ap()
    KDBG = bool(os.environ.get("KDBG"))
    if KDBG:
        d_DBG0 = nc.dram_tensor("DBG0", (P, TB + 3), BF16, kind="ExternalOutput").ap()
        d_DBG1 = nc.dram_tensor("DBG1", (P, HH), BF16, kind="ExternalOutput").ap()
        d_DBG2 = nc.dram_tensor("DBG2", (P, HH + D_STATE), BF16, kind="ExternalOutput").ap()
        d_DBG3 = nc.dram_tensor("DBG3", (KD, TB), BF16, kind="ExternalOutput").ap()
        d_DBG4 = nc.dram_tensor("DBG4", (P, NH * CH), BF16, kind="ExternalOutput").ap()
        d_DBG5 = nc.dram_tensor("DBG5", (D_STATE, HH), BF16, kind="ExternalOutput").ap()
        d_DBG6 = nc.dram_tensor("DBG6", (P, CPB * NH), FP32, kind="ExternalOutput").ap()
        d_DBG7 = nc.dram_tensor("DBG7", (KD, CPB * NH * CH), BF16, kind="ExternalOutput").ap()
        d_DBG8 = nc.dram_tensor("DBG8", (P, NH), BF16, kind="ExternalOutput").ap()
        d_DBG9 = nc.dram_tensor("DBG9", (P, NH * D_STATE), BF16, kind="ExternalOutput").ap()
        d_DBG10 = nc.dram_tensor("DBG10", (P, HH), FP32, kind="ExternalOutput").ap()
        d_DBG11 = nc.dram_tensor("DBG11", (D_STATE, HH), BF16, kind="ExternalOutput").ap()
        d_DBG12 = nc.dram_tensor("DBG12", (P, HH), BF16, kind="ExternalOutput").ap()
        d_DBG13 = nc.dram_tensor("DBG13", (P, D_MODEL), BF16, kind="ExternalOutput").ap()
        d_DBG14 = nc.dram_tensor("DBG14", (P, 2 * 384), BF16, kind="ExternalOutput").ap()
        d_DBG15 = nc.dram_tensor("DBG15", (D_STATE, NH * D_STATE), BF16, kind="ExternalOutput").ap()

    with tile.TileContext(nc, trace_sim=False) as tc, ExitStack() as ctx:
        const = ctx.enter_context(tc.tile_pool(name="const", bufs=1))
        wgt = ctx.enter_context(tc.tile_pool(name="wgt", bufs=1))
        seqp = ctx.enter_context(tc.tile_pool(name="seqp", bufs=1))
        blk1 = ctx.enter_context(tc.tile_pool(name="blk1", bufs=2))
        xbcp = ctx.enter_context(tc.tile_pool(name="xbcp", bufs=2))
        xstp = ctx.enter_context(tc.tile_pool(name="xstp", bufs=2))
        dtp = ctx.enter_context(tc.tile_pool(name="dtp", bufs=2))
        stg = ctx.enter_context(tc.tile_pool(name="stg", bufs=1))
        chk = ctx.enter_context(tc.tile_pool(name="chk", bufs=2))
        st = ctx.enter_context(tc.tile_pool(name="st", bufs=2))
        # PSUM (8 banks of 2KB): psA 2 + psD 2 + psS 4, all [128,512] slots
        psA = ctx.enter_context(tc.tile_pool(name="psA", bufs=2, space="PSUM"))
        psD = ctx.enter_context(tc.tile_pool(name="psD", bufs=2, space="PSUM"))
        psS = ctx.enter_context(tc.tile_pool(name="psS", bufs=4, space="PSUM"))

        # ---- weights: packed DMAs, two queues ----
        wcxp = wgt.tile([P, NWX], BF16, tag="wcxp")
        h = NWX // 2
        nc.sync.dma_start(wcxp[:, 0:h], d_WCX[:, 0:h])
        nc.sync.dma_start(wcxp[:, h:], d_WCX[:, h:])
        wctp = wgt.tile([P, NCT], FP32R, tag="wctp")
        nc.sync.dma_start(wctp[:], d_WCT)
        dgp = wgt.tile([P, NDG], BF16, tag="dgp")
        nc.gpsimd.dma_start(dgp[:], d_DGP)
        wtp = wgt.tile([P, NWT], BF16, tag="wtp")
        nc.gpsimd.dma_start(wtp[:], d_WTP)
        wbp = wgt.tile([P, NWB], BF16, tag="wbp")
        nc.gpsimd.dma_start(wbp[:], d_WBP)
        wcx = [wcxp[:, kt * HH:(kt + 1) * HH] for kt in range(NKT)]
        wct = [wctp[:, kt * 44:(kt + 1) * 44] for kt in range(NKT)]
        wt = [wtp[:, kt * HH:(kt + 1) * HH] for kt in range(NKT)]
        diagw = [[dgp[:, (k * NKT + ct) * P:(k * NKT + ct + 1) * P]
                  for ct in range(NKT)] for k in range(D_CONV)]
        dgb0 = D_CONV * NKT * P
        diagb = [dgp[:, dgb0 + k * D_STATE:dgb0 + (k + 1) * D_STATE]
                 for k in range(D_CONV)]
        dgc0 = dgb0 + D_CONV * D_STATE
        diagc = [dgp[:, dgc0 + k * D_STATE:dgc0 + (k + 1) * D_STATE]
                 for k in range(D_CONV)]
        wcomb = [wbp[:, ct * D_MODEL:(ct + 1) * D_MODEL] for ct in range(NKT)]
        dpid = wbp[:, NKT * D_MODEL:NKT * D_MODEL + NH * P]

        # ---- constants (vector queue; gated after TB0 x loads) ----
        const_dmas = []
        tri = const.tile([P, P], FP32)
        const_dmas.append(nc.scalar.dma_start(tri[:], d_TRI))
        convbx = const.tile([P, NKT], FP32)
        const_dmas.append(nc.scalar.dma_start(convbx[:], d_CONVBX))
        convbb = const.tile([D_STATE, 1], FP32)
        const_dmas.append(nc.scalar.dma_start(convbb[:], d_CONVBB))
        convbc = const.tile([D_STATE, 1], FP32)
        const_dmas.append(nc.scalar.dma_start(convbc[:], d_CONVBC))
        dtbias = const.tile([NH, 1], FP32)
        const_dmas.append(nc.scalar.dma_start(dtbias[:], d_DTBIAS))
        aneg = const.tile([NH, 1], FP32)
        const_dmas.append(nc.scalar.dma_start(aneg[:], d_ANEG))
        aneg_bc = const.tile([P, NH], FP32)
        const_dmas.append(nc.scalar.dma_start(aneg_bc[:], d_ANEG_BC))
        eyerep = const.tile([D_STATE, NH * D_STATE], BF16)
        const_dmas.append(nc.scalar.dma_start(eyerep[:], d_EYEREP))
        idn = const.tile([P, P], FP32); make_identity(nc, idn)
        idnr = const.tile([P, P], FP32R); nc.vector.tensor_copy(idnr[:], idn[:])

        ssqall = seqp.tile([P, NCHUNK], FP32)
        # persistent decay-staging tiles; static rows loaded once
        # lhsD rows: [0..3) ones | [3..39) = ld_j[h, s] at row 3 + h*3 + j
        # rhsD rows: [0..3) = ac_j[h, t] at col h*TB + t | [3..39) blocked ones
        lhsD = stg.tile([KD, TB], BF16, tag="lhsD")
        const_dmas.append(nc.scalar.dma_start(lhsD[0:3, :], d_ONES3))
        rhsD = stg.tile([KD, CPB * NH * CH], BF16, tag="rhsD")
        const_dmas.append(nc.scalar.dma_start(rhsD[3:, :], d_RHSC))
        hN = None
        xbc = None
        last_exp = None          # last lall exp of the previous TB
        last_sil = None          # last silu of the current TB
        expac_inst = None        # expac of the previous TB

        for tb in range(NTB):
            last_sil_prev = last_sil
            t0 = tb * TB
            xtb = []
            xtbb = []
            xdmas = []
            for kt in range(NKT):
                x = blk1.tile([P, TB], FP32R, tag=f"xtb{kt}")
                xdmas.append(nc.sync.dma_start(x[:], d_xT[kt * P:(kt + 1) * P, t0:t0 + TB]))
                xtb.append(x)
                xb16 = blk1.tile([P, TB], BF16, tag=f"xtbb{kt}")
                xdmas.append(nc.sync.dma_start(xb16[:], d_xTb[kt * P:(kt + 1) * P, t0:t0 + TB]))
                xtbb.append(xb16)
            if tb == 0:
                _after(const_dmas, *xdmas)

            # ---- in_proj c-major (conv input tiles, left-pad 3) ----
            # ct=NKT (dt|B|C rows) first so the dt chain starts early
            xbc_prev = xbc if tb > 0 else None
            xbc = [None] * (NKT + 1)
            dtC = blk1.tile([NH, TB], FP32, tag="dtC")
            for ct in [NKT] + list(range(NKT)):
                cw = P if ct < NKT else CMJ - NKT * P   # 44 in last tile
                p = psA.tile([P, 512], FP32, tag="psA")
                for kt in range(NKT):
                    if ct == NKT:
                        nc.tensor.matmul(p[:cw, 0:TB], wct[kt][:, 0:cw], xtb[kt][:],
                                         start=(kt == 0), stop=(kt == NKT - 1))
                    else:
                        nc.tensor.matmul(p[:cw, 0:TB], wcx[kt][:, ct * P:(ct + 1) * P],
                                         xtbb[kt][:],
                                         start=(kt == 0), stop=(kt == NKT - 1))
                xb = xbcp.tile([P, TB + 3], BF16, tag=f"xbc{ct}")
                if xbc_prev is None:
                    nc.vector.memset(xb[:cw, 0:3], 0.0)
                else:
                    nc.vector.tensor_copy(xb[:cw, 0:3], xbc_prev[ct][:cw, TB:TB + 3])
                nc.scalar.copy(xb[:cw, 3:], p[:cw, 0:TB])
                if ct == NKT:  # dt rows 0..11 of this tile, exact fp32 from psum
                    nc.vector.tensor_copy(dtC[:], p[0:NH, 0:TB])
                xbc[ct] = xb

            # ---- c-major dt chain -> decay staging (scheduled into the
            # PREVIOUS TB's exp block so pd/min/exp are ready at chunk start)
            spce = dtp.tile([NH, TB], FP32, tag="spce")
            spce_inst = nc.scalar.activation(spce[:], dtC[:], AF.Exp,
                                             bias=dtbias[:], scale=1.0)
            _after([spce_inst], last_sil_prev)
            spc = dtp.tile([NH, TB], FP32, tag="spc")   # = dt (softplus)
            spc_inst = nc.scalar.activation(spc[:], spce[:], AF.Ln, bias=1.0)
            ldc = dtp.tile([NH, TB], FP32, tag="ldc")
            ldc_inst = nc.scalar.activation(ldc[:], spc[:], AF.Ln)
            # pin the Ln pair to the end of the previous exp block
            _after([spc_inst, ldc_inst], last_sil_prev, last_exp)
            W = CPB * NH
            spt = dtp.tile([P, W], FP32, tag="spt")
            for i in range(CPB):
                pa = psA.tile([P, 512], FP32, tag="psA")
                nc.tensor.transpose(pa[:, 0:NH], spc[:, i * P:(i + 1) * P],
                                    idn[0:NH, 0:NH])
                nc.vector.tensor_copy(spt[:, i * NH:(i + 1) * NH], pa[:, 0:NH])
            logda = dtp.tile([P, W], FP32, tag="logda")
            nc.vector.tensor_tensor(logda[:], spt[:],
                                    _rep(aneg_bc, None, CPB, NH, 0, 1), ALU.mult)
            acum = dtp.tile([P, W], FP32, tag="acum")
            for i in range(CPB):
                pa = psA.tile([P, 512], FP32, tag="psA")
                nc.tensor.matmul(pa[:, 0:NH], tri[:], logda[:, i * NH:(i + 1) * NH],
                                 start=True, stop=True)
                nc.vector.tensor_copy(acum[:, i * NH:(i + 1) * NH], pa[:, 0:NH])
            acumC = dtp.tile([NH, TB], FP32, tag="acumC")
            for i in range(CPB):
                pcc = psA.tile([P, 512], FP32, tag="psA")
                nc.tensor.transpose(pcc[:NH, 0:P], acum[:, i * NH:(i + 1) * NH],
                                    idn[0:P, 0:P])
                nc.vector.tensor_copy(acumC[:, i * P:(i + 1) * P], pcc[:NH, 0:P])
            nc.vector.tensor_sub(ldc[:], ldc[:], acumC[:])
            # 3-way bf16 splits, j-blocks as columns: spl[:, j*TB:(j+1)*TB]
            spl = {}
            for nm, src in (("ac", acumC), ("ld", ldc)):
                s3 = dtp.tile([NH, 3 * TB], BF16, tag=nm + "3")
                r1 = dtp.tile([NH, TB], FP32, tag=nm + "r1")
                r2 = dtp.tile([NH, TB], FP32, tag=nm + "r2")
                nc.vector.tensor_copy(s3[:, 0:TB], src[:])
                nc.vector.tensor_sub(r1[:], src[:], s3[:, 0:TB])
                nc.vector.tensor_copy(s3[:, TB:2 * TB], r1[:])
                nc.vector.tensor_sub(r2[:], r1[:], s3[:, TB:2 * TB])
                nc.vector.tensor_copy(s3[:, 2 * TB:3 * TB], r2[:])
                spl[nm] = s3

            # ---- whole-TB decay staging (4 HWDGE DMAs) ----
            nc.gpsimd.dma_start(
                lhsD[3:, :],
                bass.AP(tensor=spl["ld"].tensor, offset=spl["ld"].offset,
                        ap=[[spl["ld"].ap[0][0], NH], [TB, 3], [1, TB]]))
            for j in range(3):
                for i in range(CPB):
                    dst = bass.AP(tensor=rhsD.tensor,
                                  offset=rhsD.offset + j * rhsD.ap[0][0] + i * NH * CH,
                                  ap=[[rhsD.ap[0][0], 1], [CH, NH], [1, CH]])
                    nc.gpsimd.dma_start(
                        dst, spl["ac"][:, j * TB + i * CH:j * TB + (i + 1) * CH])

            # ---- in_proj t-major: z (-> psum, silu'd later) ----
            zps = []
            for tt in range(CPB):
                for nb in range(2):
                    pz = psD.tile([P, 512], FP32, tag="psD")
                    for kt in range(NKT):
                        nc.tensor.matmul(pz[:, 0:384], xtbb[kt][:, tt * P:(tt + 1) * P],
                                         wt[kt][:, nb * 384:(nb + 1) * 384],
                                         start=(kt == 0), stop=(kt == NKT - 1))
                    zps.append(pz)

            # ---- conv (diag matmuls) ----
            convps = []
            for ct in range(NKT):
                p = psA.tile([P, 384], FP32, tag="psA")
                for k in range(D_CONV):
                    nc.tensor.matmul(p[:, 0:TB], diagw[k][ct][:], xbc[ct][:, k:k + TB],
                                     start=(k == 0), stop=(k == D_CONV - 1))
                convps.append(p)
            bcps = []
            for dg in (diagb, diagc):
                p = psA.tile([P, 384], FP32, tag="psA")
                for k in range(D_CONV):
                    nc.tensor.matmul(p[:D_STATE, 0:TB], dg[k][0:44, :], xbc[NKT][0:44, k:k + TB],
                                     start=(k == 0), stop=(k == D_CONV - 1))
                bcps.append(p)

            # ---- silu block (single act-table load); sz first to free psD ----
            sil_insts = []
            sztiles = []
            for tt in range(CPB):
                sz = dtp.tile([P, HH], BF16, tag=f"sz{tt}")
                sil_insts.append(
                    nc.scalar.activation(sz[:, 0:384], zps[2 * tt][:, 0:384], AF.Silu))
                sil_insts.append(
                    nc.scalar.activation(sz[:, 384:HH], zps[2 * tt + 1][:, 0:384], AF.Silu))
                sztiles.append(sz)
            xsil = []
            for ct in range(NKT):
                xsl = blk1.tile([P, TB], FP32R, tag=f"xsil{ct}")
                sil_insts.append(
                    nc.scalar.activation(xsl[:], convps[ct][:, 0:TB], AF.Silu,
                                         bias=convbx[:, ct:ct + 1], scale=1.0))
                xsil.append(xsl)
            bsil = blk1.tile([D_STATE, TB], BF16, tag="bsil")
            csil = blk1.tile([D_STATE, TB], BF16, tag="csil")
            bsilF = blk1.tile([D_STATE, TB], FP32R, tag="bsilF")
            sil_insts.append(
                nc.scalar.activation(bsilF[:], bcps[0][:D_STATE, 0:TB], AF.Silu,
                                     bias=convbb[:], scale=1.0))
            for dst, pp, bias in ((bsil, bcps[0], convbb), (csil, bcps[1], convbc)):
                sil_insts.append(
                    nc.scalar.activation(dst[:], pp[:D_STATE, 0:TB], AF.Silu,
                                         bias=bias[:], scale=1.0))
            _after(sil_insts, last_exp, expac_inst, ldc_inst)
            last_sil = sil_insts[-1]

            # ---- transpose x + B to s-major (bf16) ----
            xs_tiles = []
            for tt in range(CPB):
                xst = xstp.tile([P, HH + D_STATE], BF16, tag=f"xst{tt}")
                for g in range(2):
                    pt = psA.tile([P, 512], FP32, tag="psA")
                    for k in range(3):
                        ct = g * 3 + k
                        nc.tensor.transpose(pt[:, k * P:(k + 1) * P].bitcast(FP32R),
                                            xsil[ct][:, tt * P:(tt + 1) * P], idnr[:])
                    nc.vector.tensor_copy(xst[:, g * 384:(g + 1) * 384], pt[:, 0:384])
                ptb = psA.tile([P, 512], FP32, tag="psA")
                nc.tensor.transpose(ptb[:, 0:D_STATE].bitcast(FP32R),
                                    bsilF[:, tt * P:(tt + 1) * P],
                                    idnr[0:D_STATE, 0:D_STATE])
                nc.vector.tensor_copy(xst[:, HH:HH + D_STATE], ptb[:, 0:D_STATE])
                xs_tiles.append(xst)

            # ---- expac for the chunk epilogue gate ----
            expac = dtp.tile([P, W], FP32, tag="expac")
            expac_inst = nc.scalar.activation(expac[:], acum[:], AF.Exp)
            _after([expac_inst], last_sil)
            expacB = dtp.tile([P, W], BF16, tag="expacB")
            nc.vector.tensor_copy(expacB[:], expac[:])
            eaebc = dtp.tile([D_STATE, W], BF16, tag="eaebc")
            nc.gpsimd.dma_start(eaebc[:], _pbcast(expacB[P - 1:P, :], D_STATE))
            aendbc = dtp.tile([P, W], FP32, tag="aendbc")
            nc.gpsimd.dma_start(aendbc[:], _pbcast(acum[P - 1:P, :], P))

            if KDBG and tb == 0:
                nc.sync.dma_start(d_DBG1, sztiles[0][:])
                nc.sync.dma_start(d_DBG2, xs_tiles[0][:])
                nc.sync.dma_start(d_DBG3, lhsD[:])
                nc.sync.dma_start(d_DBG6, expac[:])
                nc.sync.dma_start(d_DBG7, rhsD[:])
            # ---- chunks ----
            for i in range(CPB):
                ci = (t0 // P) + i
                xst = xs_tiles[i]
                acs = slice(i * NH, (i + 1) * NH)
                ccs = slice(i * P, (i + 1) * P)

                # D-plane: pd_nb covers h in [4nb, 4nb+4)
                lall = chk.tile([P, NH * CH], BF16, tag="lall")
                lmin = chk.tile([P, NH * CH], FP32, tag="lmin")
                for nb in range(3):
                    pdm = psD.tile([P, 512], FP32, tag="psD")
                    c0 = i * NH * CH + nb * 512
                    nc.tensor.matmul(pdm[:], lhsD[:, ccs], rhsD[:, c0:c0 + 512],
                                     start=True, stop=True)
                    sl = slice(nb * 512, (nb + 1) * 512)
                    nc.vector.tensor_scalar_min(lmin[:, sl], pdm[:], 25.0)
                    last_exp = nc.scalar.activation(lall[:, sl], lmin[:, sl], AF.Exp)
                    _after([last_exp], last_sil)

                pcbt = psS.tile([P, 512], FP32, tag="psS")
                nc.tensor.matmul(pcbt[:, 0:P], bsil[:, ccs], csil[:, ccs],
                                 start=True, stop=True)
                cbtm = chk.tile([P, P], BF16, tag="cbtm")
                nc.vector.tensor_tensor(cbtm[:], pcbt[:, 0:P], tri[:], ALU.mult)
                mall = chk.tile([P, NH * CH], BF16, tag="mall")
                nc.vector.tensor_tensor(mall[:], _rep(cbtm, None, NH, CH, 0, 1),
                                        lall[:], ALU.mult)

                # ws[s,h] = exp((logdt_s - Acum_s) + Aend)
                ptw = psA.tile([P, 512], FP32, tag="psA")
                nc.tensor.transpose(ptw[:, 0:NH], ldc[:, ccs], idn[0:NH, 0:NH])
                wsf = chk.tile([P, NH], FP32, tag="wsf")
                nc.vector.tensor_tensor(wsf[:], ptw[:, 0:NH],
                                        aendbc[:, acs], ALU.add)
                ws = chk.tile([P, NH], BF16, tag="ws")
                ws_exp = nc.scalar.activation(ws[:], wsf[:], AF.Exp)
                _after([ws_exp], last_sil)
                bd = chk.tile([P, NH * D_STATE], BF16, tag="bd")
                nc.vector.tensor_tensor(
                    bd[:],
                    bass.AP(tensor=xst.tensor, offset=xst.offset + HH,
                            ap=[[xst.ap[0][0], P], [0, NH], [1, D_STATE]]),
                    bass.AP(tensor=ws.tensor, offset=ws.offset,
                            ap=[[ws.ap[0][0], P], [1, NH], [0, D_STATE]]),
                    ALU.mult)

                if KDBG and ci == 0:
                    nc.sync.dma_start(d_DBG4, lall[:])
                hN_prev = hN
                # y2 = C @ hN (gated by expac in epilogue)
                py2 = [None, None]
                if hN_prev is not None:
                    for g, (c0, cw) in enumerate(((0, 512), (512, 256))):
                        p2 = psS.tile([P, 512], FP32, tag="psS")
                        nc.tensor.matmul(p2[:, 0:cw], csil[:, ccs],
                                         hN_prev[:, c0:c0 + cw], start=True, stop=True)
                        py2[g] = p2

                # state update: pst = sum_h [eae_h * I16 ; bd_h] @ [hN_h ; x_h]
                eyes = None
                if hN_prev is not None:
                    eyes = chk.tile([D_STATE, NH * D_STATE], BF16, tag="eyes")
                    nc.vector.tensor_tensor(
                        eyes[:], eyerep[:],
                        bass.AP(tensor=eaebc.tensor, offset=eaebc.offset + i * NH,
                                ap=[[eaebc.ap[0][0], D_STATE], [1, NH], [0, D_STATE]]),
                        ALU.mult)
                if KDBG and ci == 1 and eyes is not None:
                    nc.sync.dma_start(d_DBG15, eyes[:])
                hN_new = st.tile([D_STATE, HH], BF16, tag="hN")
                for g, (h0, nh) in enumerate(((0, 8), (8, 4))):
                    pst = psS.tile([P, 512], FP32, tag="psS")
                    for hi in range(nh):
                        h = h0 + hi
                        hs = slice(h * HEADDIM, (h + 1) * HEADDIM)
                        ls = slice(hi * HEADDIM, (hi + 1) * HEADDIM)
                        if eyes is not None:
                            nc.tensor.matmul(pst[:D_STATE, ls],
                                             eyes[:, h * D_STATE:(h + 1) * D_STATE],
                                             hN_prev[:, hs], start=True, stop=False)
                        nc.tensor.matmul(pst[:D_STATE, ls],
                                         bd[:, h * D_STATE:(h + 1) * D_STATE],
                                         xst[:, hs], start=(eyes is None), stop=True)
                    nc.vector.tensor_copy(
                        hN_new[:, h0 * HEADDIM:(h0 + nh) * HEADDIM],
                        pst[:D_STATE, 0:nh * HEADDIM])
                hN = hN_new
                if KDBG and ci == 0:
                    nc.sync.dma_start(d_DBG5, hN_new[:])
                    nc.sync.dma_start(d_DBG8, ws[:])
                    nc.sync.dma_start(d_DBG9, bd[:])
                if KDBG and ci == 1:
                    nc.sync.dma_start(d_DBG11, hN_new[:])

                # py = (Dp*I + mall) @ x
                pys = []
                for g, (h0, nh) in enumerate(((0, 8), (8, 4))):
                    py = psS.tile([P, 512], FP32, tag="psS")
                    for hi in range(nh):
                        h = h0 + hi
                        hs = slice(h * HEADDIM, (h + 1) * HEADDIM)
                        ls = slice(hi * HEADDIM, (hi + 1) * HEADDIM)
                        nc.tensor.matmul(py[:, ls], dpid[:, h * P:(h + 1) * P],
                                         xst[:, hs], start=True, stop=False)
                        nc.tensor.matmul(py[:, ls], mall[:, h * CH:(h + 1) * CH],
                                         xst[:, hs], start=False, stop=True)
                    pys.append(py)

                # epilogue: yg = (py2*expac + py) * silu(z)
                yg = chk.tile([P, HH], FP32R, tag="yg")
                if hN_prev is not None:
                    e1 = chk.tile([P, HH], FP32, tag="e1")
                    for g, (h0, nh) in enumerate(((0, 8), (8, 4))):
                        cs = slice(h0 * HEADDIM, (h0 + nh) * HEADDIM)
                        nc.vector.tensor_tensor(
                            e1[:, cs], py2[g][:, 0:nh * HEADDIM],
                            bass.AP(tensor=expac.tensor,
                                    offset=expac.offset + i * NH + h0,
                                    ap=[[expac.ap[0][0], P], [1, nh], [0, HEADDIM]]),
                            ALU.mult)
                        nc.vector.tensor_tensor(e1[:, cs], e1[:, cs],
                                                pys[g][:, 0:nh * HEADDIM], ALU.add)
                    nc.vector.tensor_tensor(yg[:], e1[:], sztiles[i][:], ALU.mult)
                else:
                    for g, (h0, nh) in enumerate(((0, 8), (8, 4))):
                        cs = slice(h0 * HEADDIM, (h0 + nh) * HEADDIM)
                        nc.vector.tensor_tensor(yg[:, cs], pys[g][:, 0:nh * HEADDIM],
                                                sztiles[i][:, cs], ALU.mult)
                if KDBG and ci == 0:
                    nc.sync.dma_start(d_DBG10, yg[:].bitcast(FP32))
                if KDBG and ci == 1:
                    nc.sync.dma_start(d_DBG12, sztiles[i][:])
                sqs = chk.tile([P, HH], FP32, tag="sqs")
                nc.scalar.activation(sqs[:], yg[:], AF.Square,
                                     accum_out=ssqall[:, ci:ci + 1])

                ygts = []
                for g in range(2):
                    ptr = psS.tile([P, 512], FP32, tag="psS")
                    for k in range(3):
                        ct = g * 3 + k
                        nc.tensor.transpose(ptr[:, k * P:(k + 1) * P].bitcast(FP32R),
                                            yg[:, ct * P:(ct + 1) * P], idnr[:])
                    ygt = chk.tile([P, 384], BF16, tag=f"ygt{g}")
                    nc.vector.tensor_copy(ygt[:], ptr[:, 0:384])
                    ygts.append(ygt)
                o1 = chk.tile([P, D_MODEL], BF16, tag="o1")
                for g, (c0, cw) in enumerate(((0, 512), (512, 256))):
                    pw = psS.tile([P, 512], FP32, tag="psS")
                    for ct in range(NKT):
                        ygt_sl = ygts[ct // 3][:, (ct % 3) * P:(ct % 3 + 1) * P]
                        nc.tensor.matmul(pw[:, 0:cw], ygt_sl, wcomb[ct][:, c0:c0 + cw],
                                         start=(ct == 0), stop=(ct == NKT - 1))
                    nc.vector.tensor_copy(o1[:, c0:c0 + cw], pw[:, 0:cw])
                if KDBG and ci == 0:
                    nc.sync.dma_start(d_DBG13, o1[:])
                    nc.sync.dma_start(d_DBG14[:, 0:384], ygts[0][:])
                    nc.sync.dma_start(d_DBG14[:, 384:768], ygts[1][:])
                nc.sync.dma_start(d_OUT1[ci * P:(ci + 1) * P, :], o1[:])

        nc.sync.dma_start(d_OUT2, ssqall[:])

    nc.compile()
    return nc


# ================= host side =================

def _prep_core_inputs(x_b_T, in_w, conv_w, conv_b, dt_bias, A_log, Dp, norm_w,
                      out_w, proj_w_dir, hh):
    import ml_dtypes
    D_INNER = 1536
    zsel = slice(hh * HH, (hh + 1) * HH)
    xsel = slice(D_INNER + hh * HH, D_INNER + (hh + 1) * HH)
    Bsel = slice(2 * D_INNER, 2 * D_INNER + 16)
    Csel = slice(2 * D_INNER + 16, 2 * D_INNER + 32)
    dtsel = slice(2 * D_INNER + 32 + hh * NH, 2 * D_INNER + 32 + (hh + 1) * NH)

    # c-major rows: [x 768 | dt 12 | B 16 | C 16]
    Wc_rows = np.concatenate([in_w[xsel], in_w[dtsel], in_w[Bsel], in_w[Csel]], 0)
    Wt_rows = in_w[zsel]

    cwx = conv_w[hh * HH:(hh + 1) * HH]          # (768, 4) x-part
    cbx = conv_b[hh * HH:(hh + 1) * HH]
    cwB = conv_w[D_INNER:D_INNER + 16]
    cbB = conv_b[D_INNER:D_INNER + 16]
    cwC = conv_w[D_INNER + 16:D_INNER + 32]
    cbC = conv_b[D_INNER + 16:D_INNER + 32]

    DIAGW = np.zeros((D_CONV, NKT, P, P), np.float32)
    for k in range(D_CONV):
        for ct in range(NKT):
            DIAGW[k, ct][np.arange(P), np.arange(P)] = cwx[ct * P:(ct + 1) * P, k]
    DIAGB = np.zeros((D_CONV, P, D_STATE), np.float32)
    DIAGC = np.zeros((D_CONV, P, D_STATE), np.float32)
    for k in range(D_CONV):
        DIAGB[k][NH + np.arange(16), np.arange(16)] = cwB[:, k]       # in-rows 12..27
        DIAGC[k][NH + 16 + np.arange(16), np.arange(16)] = cwC[:, k]  # in-rows 28..43
    CONVBX = np.zeros((P, NKT), np.float32)
    for ct in range(NKT):
        CONVBX[:, ct] = cbx[ct * P:(ct + 1) * P]

    # p-major packs (one contiguous DMA each); WCP mixes bf16 x-rows with
    # fp32 dt|B|C rows, byte-packed into fp32 words
    Wc_T = Wc_rows.T.astype(np.float32)               # (768, 812)
    Wt_T = Wt_rows.T.astype(np.float32)               # (768, 768)
    WCX = np.concatenate([Wc_T[kt * P:(kt + 1) * P, 0:768] for kt in range(NKT)], 1)
    WCT = np.concatenate([Wc_T[kt * P:(kt + 1) * P, 768:812] for kt in range(NKT)], 1)
    WTP = np.concatenate([Wt_T[kt * P:(kt + 1) * P] for kt in range(NKT)], 1)
    DGP = np.concatenate(
        [DIAGW[k, ct] for k in range(D_CONV) for ct in range(NKT)]
        + [DIAGB[k] for k in range(D_CONV)] + [DIAGC[k] for k in range(D_CONV)], 1)

    a_neg = -np.exp(A_log[hh * NH:(hh + 1) * NH]).astype(np.float32)
    dtb = dt_bias[hh * NH:(hh + 1) * NH].astype(np.float32)
    TRIm = np.triu(np.ones((P, P), np.float32))
    RHSC = np.zeros((3 * NH, CPB * NH * CH), np.float32)
    for h in range(NH):
        for j in range(3):
            for i in range(CPB):
                RHSC[h * 3 + j, i * NH * CH + h * CH:i * NH * CH + (h + 1) * CH] = 1.0
    EYEREP = np.zeros((D_STATE, NH * D_STATE), np.float32)
    for h in range(NH):
        EYEREP[np.arange(16), h * D_STATE + np.arange(16)] = 1.0
    DPID = np.zeros((P, NH * P), np.float32)
    dp = Dp[hh * NH:(hh + 1) * NH].astype(np.float32)
    for h in range(NH):
        DPID[np.arange(P), h * P + np.arange(P)] = dp[h]
    ow = (out_w * norm_w[None, :]).astype(np.float32)
    WCOMB = np.ascontiguousarray((proj_w_dir @ ow)[:, hh * HH:(hh + 1) * HH].T)
    WBP = np.concatenate(
        [WCOMB[ct * P:(ct + 1) * P] for ct in range(6)] + [DPID], 1)

    bf = lambda a: np.ascontiguousarray(a).astype(ml_dtypes.bfloat16)
    f = np.ascontiguousarray
    return {
        "xT": f(x_b_T.astype(np.float32)),
        "xTb": bf(x_b_T),
        "WCX": bf(WCX), "WCT": f(WCT), "WTP": bf(WTP), "DGP": bf(DGP), "WBP": bf(WBP),
        "CONVBX": CONVBX,
        "CONVBB": f(cbB.astype(np.float32)[:, None]),
        "CONVBC": f(cbC.astype(np.float32)[:, None]),
        "DTBIAS": f(dtb[:, None]),
        "ANEG": f(a_neg[:, None]),
        "ANEG_BC": f(np.repeat(a_neg[None, :], P, 0)),
        "TRI": TRIm,
        "ONES3": bf(np.ones((3, TB), np.float32)),
        "RHSC": bf(RHSC),
        "EYEREP": bf(EYEREP),
    }


def make_in_maps(inputs):
    x = np.asarray(inputs["x"], np.float32)
    proj_w = np.asarray(inputs["proj_w"], np.float32)
    in_maps, core_meta = [], []
    for b in range(2):
        for d, pref in ((0, "f_"), (1, "b_")):
            xb = x[b] if d == 0 else x[b][::-1]
            for hh in range(2):
                g = lambda n: np.asarray(inputs[pref + n], np.float32)
                im = _prep_core_inputs(
                    np.ascontiguousarray(xb.T), g("in_w"), g("conv_w"), g("conv_b"),
                    g("dt_bias"), g("A_log"), g("Dp"), g("norm_w"), g("out_w"),
                    proj_w[:, d * D_MODEL:(d + 1) * D_MODEL], hh)
                in_maps.append(im)
                core_meta.append((b, d, hh))
    return in_maps, core_meta


def combine_outputs(results, core_meta, proj_b):
    out = np.zeros((2, SEQ, D_MODEL), np.float32)
    for b in range(2):
        for d in range(2):
            idx = [i for i, (bb, dd, _) in enumerate(core_meta) if bb == b and dd == d]
            part = sum(np.asarray(results[i]["OUT1"], np.float32) for i in idx)
            ssq = sum(np.asarray(results[i]["OUT2"], np.float32) for i in idx)
            ssq_t = ssq.T.reshape(SEQ)                        # t = ci*128 + p
            s = 1.0 / np.sqrt(ssq_t / 1536.0 + EPS)
            contrib = part * s[:, None]
            if d == 1:
                contrib = contrib[::-1]
            out[b] += contrib
    out += np.asarray(proj_b, np.float32)[None, None, :]
    return out


_NC_CACHE = {}


def kernel(**inputs):
    in_maps, core_meta = make_in_maps(inputs)
    if "nc" not in _NC_CACHE:
        _NC_CACHE["nc"] = build_program()
    nc = _NC_CACHE["nc"]
    res = run_bass_kernel_spmd(nc, in_maps, list(range(8)))
    return combine_outputs(res.results, core_meta, inputs["proj_b"])
